# revision 3
# baseline (speedup 1.0000x reference)
"""Trainium2 Bass kernel for nn_BaseImplicitConv — fully on-device version.

out = fft_conv(u, filt) * (u @ pw^T + pb) + u,  filt = MLP(pos_emb)

8 cores = 4 batches x 2 d-halves. Each core receives u[b] (bf16) and
computes everything on device: the filter MLP tail (h @ w2 + b2), the
causal FFT convolution for its 512 channels (matmul-based 4-step DFT,
N = 8192 = 64 x 128), the d_model x d_model projection for its column
half, and the elementwise gate + residual. Host only computes the tiny
MLP hidden layer h = relu(pe @ w1^T + b1) and re-assembles the output.

DFT factorization (N = 8192, N1 = 64, N2 = 128), per channel x:
  A[n1, n2] = x[64 n2 + n1]            (zero for n2 >= 64  => skip)
  B[n1, k2] = sum_{n2<64} A[n1, n2] W128^{n2 k2}
  C[n1, k2] = B[n1, k2] * W8192^{n1 k2}
  X[k1, k2] = sum_{n1} W64^{n1 k1} C[n1, k2]      (k = 128 k1 + k2)
Inverse (y real, only l < 4096 needed; l = 64 q + s):
  Z[s, k2]  = sum_{k1} W64^{-s k1} Y[k1, k2]
  Z2[s, k2] = Z[s, k2] * W8192^{-s k2} / 8192
  y[64q+s]  = Re sum_{k2} Z2[s, k2] W128^{-q k2}
Channels are processed two-per-matmul via block-diagonal DFT matrices
(partition dim = (channel parity, n1|s|k1)).
"""

import math
import sys

import numpy as np
import ml_dtypes

sys.path.insert(0, "/opt/trn_rl_repo")
sys.path.insert(0, "/opt/trn_rl_repo/concourse")

import concourse.bass as bass
import concourse.mybir as mybir
from concourse import tile
from concourse.vector_clock import ScopedClock
import bass_rust

B, L, D = 4, 4096, 1024
N_CORES = 8
HALF = D // 2          # 512 columns per core
PASS_C = 128           # channels per pass
N_PASS = HALF // PASS_C
CHUNK_C = 16           # channels per conv chunk
N_CHUNK = PASS_C // CHUNK_C
NPAIR = CHUNK_C // 2   # 8 channel-pairs per chunk

F32 = mybir.dt.float32
BF16 = mybir.dt.bfloat16
NPBF = ml_dtypes.bfloat16


def _patch_tile_drain():
    """walrus in this container rejects >1 sync-wait on a CTRL (Drain)
    instruction; emit each wait on its own NOP instead."""

    def _drain_and_barrier(self, tick_clock, wait_clock):
        drain_inst = self.nc.sync.drain()
        wait_clock.add_sem_waits(
            drain_inst.ins, ScopedClock({None: tick_clock.global_clock})
        )
        si = drain_inst.ins.sync_info
        if si is not None and len(si.on_wait) > 1:
            waits = list(si.on_wait)
            drain_inst.ins.sync_info = bass_rust.SyncInfo(
                on_wait=[], on_update=list(si.on_update)
            )
            for w in waits:
                wi = self.nc.sync.nop(nofuse=True)
                wi.ins.sync_info = bass_rust.SyncInfo(on_wait=[w], on_update=[])
        self.nc.all_engine_barrier()
        assert self.sems is not None
        popped = self.nc._tile_sem_poison_stack.pop()
        assert popped is self._sem_poison
        self.nc.clear_and_free_semaphores(list(self.sems.allocated().values()))
        self.nc.all_engine_barrier()

    tile.TileContext._drain_and_barrier = _drain_and_barrier


_patch_tile_drain()

_SPLIT_CTR = [0]


def _split_multi_waits(nc):
    """This walrus build allows at most one sync-wait per instruction; hoist
    extras onto same-engine NOPs placed immediately before the instruction."""
    for f in nc.m.functions:
        for bb in f.blocks:
            new_insts = []
            changed = False
            for inst in bb.instructions:
                si = inst.sync_info
                if si is not None and len(si.on_wait) > 1:
                    waits = list(si.on_wait)
                    for w in waits[:-1]:
                        _SPLIT_CTR[0] += 1
                        nop = mybir.InstNoOp(
                            name=f"wsplit-{_SPLIT_CTR[0]}", ins=[], outs=[]
                        )
                        nop.engine = inst.engine
                        nop.sync_info = bass_rust.SyncInfo(
                            on_wait=[w], on_update=[]
                        )
                        nc.register_instruction(nop, overwrite=True)
                        new_insts.append(nop)
                    inst.sync_info = bass_rust.SyncInfo(
                        on_wait=[waits[-1]], on_update=list(si.on_update)
                    )
                    changed = True
                new_insts.append(inst)
            if changed:
                bb.instructions = new_insts


def _np_consts():
    """DFT matrices / twiddles baked into the NEFF as inline consts."""
    a64 = np.arange(64)
    a128 = np.arange(128)
    c = {}
    # step1 moving: F1[n2, k2] = W128^{n2 k2} = e^{-2pi i n2 k2/128}
    th = 2.0 * np.pi * np.outer(a64, a128) / 128.0
    c["F1r"] = np.cos(th)
    c["F1i"] = -np.sin(th)
    # step3 stationary blockdiag: W64^{n1 k1} = cos - i sin
    th = 2.0 * np.pi * np.outer(a64, a64) / 64.0
    cs, sn = np.cos(th), np.sin(th)
    blk = lambda m: np.block(
        [[m, np.zeros((64, 64))], [np.zeros((64, 64)), m]]
    )
    c["F2c"] = blk(cs)       # Cr->Xr and Ci->Xi
    c["F2s"] = blk(sn)       # Ci->Xr  (real += Ci*sin)
    c["F2sn"] = blk(-sn)     # Cr->Xi  (imag += -Cr*sin)
    # inverse stepA stationary blockdiag: W64^{+s k1} = cos + i sin
    c["V1c"] = blk(cs)       # Yr->Zr and Yi->Zi
    c["V1sn"] = blk(-sn)     # Yi->Zr  (real += -Yi*sin)
    c["V1s"] = blk(sn)       # Yr->Zi
    # stepB stationary: lhsT[k2, q] = e^{+2pi i k2 q/128}: real-part only
    th = 2.0 * np.pi * np.outer(a128, a64) / 128.0
    c["V2c"] = np.cos(th)    # ZTr -> y
    c["V2sn"] = -np.sin(th)  # ZTi -> y
    c["IDT"] = np.eye(128)
    c["ONES"] = np.ones((1, 64))
    for k in list(c):
        c[k] = np.ascontiguousarray(c[k].astype(NPBF))
    # forward twiddle (f32, 4-pair replicated): e^{-2pi i n1 k2/8192}
    # layout [p=(cc,n1), j=(pr4, k2)]
    th = 2.0 * np.pi * np.outer(np.tile(a64, 2), np.tile(a128, 4)) / 8192.0
    c["TWr"] = np.cos(th).astype(np.float32)
    c["TWi"] = (-np.sin(th)).astype(np.float32)
    # inverse twiddle incl 1/N: e^{+2pi i s k2/8192}/8192
    c["TIr"] = (np.cos(th) / 8192.0).astype(np.float32)
    c["TIi"] = (np.sin(th) / 8192.0).astype(np.float32)
    return c


_NC_CACHE = {}


def _build_nc():
    if "nc" in _NC_CACHE:
        return _NC_CACHE["nc"]
    nc = bass.Bass(num_devices=N_CORES)

    # ub carries ONLY the core's 512 conv/residual channels; the
    # projection contraction over the other d-half happens on the pair
    # core and is combined via a pairwise ReduceScatter of partials.
    ub = nc.dram_tensor("ub", [L, HALF], BF16, kind="ExternalInput")
    # pwh rows = the core's d-half, columns = GLOBAL output order
    pwh = nc.dram_tensor("pwh", [HALF, D], BF16, kind="ExternalInput")
    hT = nc.dram_tensor("hT", [17, L], BF16, kind="ExternalInput")
    w2b = nc.dram_tensor("w2b", [17, HALF], BF16, kind="ExternalInput")
    # pb/2 over ALL columns (both cores contribute half via the RS sum)
    pbh = nc.dram_tensor("pbh", [1, D], BF16, kind="ExternalInput")
    out = nc.dram_tensor("out", [L, HALF], BF16, kind="ExternalOutput")
    ppart = nc.dram_tensor("ppart", [2 * L, HALF], F32, kind="Internal")
    pred = nc.dram_tensor("pred", [L, HALF], F32, kind="Internal")

    cn = {k: nc.inline_tensor(v, name=f"c_{k}") for k, v in _np_consts().items()}

    with tile.TileContext(nc) as tc:
        with (
            tc.tile_pool(name="const", bufs=1) as p_const,
            tc.tile_pool(name="inp", bufs=1) as p_inp,
            tc.tile_pool(name="uT", bufs=1) as p_uT,
            tc.tile_pool(name="stage", bufs=2) as p_stage,
            tc.tile_pool(name="persist", bufs=1) as p_pass,
            tc.tile_pool(name="work", bufs=1) as p_work,
            tc.tile_pool(name="psb", bufs=2, space="PSUM") as ps_big,
            tc.tile_pool(name="pss", bufs=2, space="PSUM") as ps_small,
        ):
            # ---- constants to SBUF ----
            ct = {}
            for k, hd in cn.items():
                shape = list(hd.shape)
                dt = BF16 if k not in ("TWr", "TWi", "TIr", "TIi") else F32
                t = p_const.tile(shape, dt, tag=f"c_{k}")
                nc.sync.dma_start(out=t[:], in_=hd[:])
                ct[k] = t

            # ---- small inputs to SBUF ----
            pw_t = p_inp.tile([128, 4 * D], BF16, tag="pw")
            nc.sync.dma_start(
                out=pw_t[:].rearrange("p (dt c) -> p dt c", dt=4),
                in_=pwh.rearrange("(dt p) c -> p dt c", p=128),
            )
            h_t = p_inp.tile([17, L], BF16, tag="h")
            nc.sync.dma_start(out=h_t[:], in_=hT[:, :])
            w2b_t = p_inp.tile([17, HALF], BF16, tag="w2b")
            nc.sync.dma_start(out=w2b_t[:], in_=w2b[:, :])
            pbh_t = p_inp.tile([1, D], BF16, tag="pbh")
            nc.sync.dma_start(out=pbh_t[:], in_=pbh[:, :])

            # ---- Phase A: build uT [d%128, (dt8, l)] via PE transposes ----
            uT = p_uT.tile([128, 4 * L], BF16, tag="uT")
            uT3 = uT[:].rearrange("p (dt l) -> p dt l", dt=4)
            for lt in range(L // 128):
                ubt = p_stage.tile([128, HALF], BF16, tag="ubt")
                nc.sync.dma_start(
                    out=ubt[:], in_=ub[lt * 128 : (lt + 1) * 128, :]
                )
                pst = ps_small.tile([128, 512], BF16, tag="smps")
                for dt_i in range(4):
                    nc.tensor.transpose(
                        pst[:, dt_i * 128 : (dt_i + 1) * 128],
                        ubt[:, dt_i * 128 : (dt_i + 1) * 128],
                        ct["IDT"][:],
                    )
                nc.scalar.copy(
                    out=uT3[:, :, lt * 128 : (lt + 1) * 128],
                    in_=pst[:].rearrange("p (j l) -> p j l", j=4),
                )

            # ---- projection partial sweep (all 1024 global cols) ----
            # lhsT = uT comb [128, 64] (d-tile, fixed s), rhs = pw cols
            uT4 = uT[:].rearrange("p (dt q s) -> p dt s q", dt=4, q=64, s=64)
            pw3 = pw_t[:].rearrange("p (dt c) -> p dt c", dt=4)
            pp3 = ppart.rearrange("(pc q s) c -> pc q s c", pc=2, s=64)
            for s in range(64):
                for pc in range(2):
                    cs = slice(pc * HALF, (pc + 1) * HALF)
                    prps = ps_small.tile([64, HALF], F32, tag="smps")
                    for dt_i in range(4):
                        nc.tensor.matmul(
                            prps[:],
                            uT4[:, dt_i, s, :],
                            pw3[:, dt_i, cs],
                            start=(dt_i == 0),
                            stop=False,
                        )
                    nc.tensor.matmul(
                        prps[:], ct["ONES"][:], pbh_t[:, cs],
                        start=False, stop=True,
                    )
                    prst = p_stage.tile([64, HALF], F32, tag="prst")
                    nc.scalar.copy(out=prst[:], in_=prps[:])
                    nc.sync.dma_start(out=pp3[pc, :, s, :], in_=prst[:])
            # pairwise sum of partials; each core keeps its own column half
            nc.gpsimd.collective_compute(
                "ReduceScatter",
                mybir.AluOpType.add,
                replica_groups=[[0, 1], [2, 3], [4, 5], [6, 7]],
                ins=[ppart[:, :]],
                outs=[pred[:, :]],
            )

            # per-pass persistent tiles
            for pp in range(N_PASS):
                c0 = pp * PASS_C  # column offset within the half

                projb = p_pass.tile([64, 64 * PASS_C], F32, tag="projb")
                ufft = p_pass.tile([64, 64 * PASS_C], BF16, tag="ufft")
                ffft = p_pass.tile([64, 64 * PASS_C], BF16, tag="ffft")

                nc.sync.dma_start(
                    out=projb[:].rearrange("p (s c) -> p s c", c=PASS_C),
                    in_=pred.rearrange("(q s) c -> q s c", s=64)[
                        :, :, c0 : c0 + PASS_C
                    ],
                )

                # ---- u in FFT layout [n2, (pr, cc, n1)] (pair-major) ----
                # conv channels are ub columns [0, 512) (host permutes the
                # core's half to the front); pass pp covers pairs
                # [pp*64, (pp+1)*64).
                nc.sync.dma_start(
                    out=ufft[:].rearrange("p (n c) -> p n c", n=64),
                    in_=ub.rearrange("(n2 n1) d -> n2 n1 d", n1=64)[
                        :, :, c0 : c0 + PASS_C
                    ],
                )

                # ---- filter values in FFT layout via h-comb matmuls ----
                h3 = h_t[:].rearrange("p (n2 n1) -> p n1 n2", n1=64)
                for n1 in range(64):
                    fps = ps_small.tile([64, PASS_C], F32, tag="smps")
                    nc.tensor.matmul(
                        fps[:],
                        h3[:, n1, :],
                        w2b_t[:, c0 : c0 + PASS_C],
                        start=True,
                        stop=True,
                    )
                    nc.scalar.copy(
                        out=ffft[:, n1 * PASS_C : (n1 + 1) * PASS_C],
                        in_=fps[:],
                    )

                # pair-major chunk views: iteration (pr, cc, n1)
                uf4 = ufft[:].rearrange(
                    "p (n pr cc) -> p pr cc n", n=64, cc=2
                )
                ff4 = ffft[:].rearrange(
                    "p (n pr cc) -> p pr cc n", n=64, cc=2
                )

                def fwd_fft(src4, ch, tag):
                    """Forward FFT of one 16-channel chunk; returns X psum
                    tiles (r, i) laid out [128=(cc,k1), (pr8, k2)]."""
                    # gather the chunk into a pair-contiguous tile:
                    # [n2, (pr8, cc2, n1-64)]
                    pairt = p_work.tile([64, 1024], BF16, tag=f"pair{tag}")
                    nc.gpsimd.tensor_copy(
                        out=pairt[:],
                        in_=src4[:, ch * NPAIR : (ch + 1) * NPAIR, :, :],
                    )
                    # step1: B[(cc,n1), (pr, k2)]
                    br = ps_big.tile([128, 1024], F32, tag="bigps")
                    bi = ps_big.tile([128, 1024], F32, tag="bigps")
                    for p in range(NPAIR):
                        st = pairt[:, p * 128 : (p + 1) * 128]
                        nc.tensor.matmul(
                            br[:, p * 128 : (p + 1) * 128], st, ct["F1r"][:],
                            start=True, stop=True,
                        )
                        nc.tensor.matmul(
                            bi[:, p * 128 : (p + 1) * 128], st, ct["F1i"][:],
                            start=True, stop=True,
                        )
                    # twiddle: C = B * W8192^{n1 k2}  (vector+gpsimd)
                    cr = p_work.tile([128, 1024], BF16, tag=f"C{tag}r")
                    ci = p_work.tile([128, 1024], BF16, tag=f"C{tag}i")
                    t1 = p_work.tile([128, 512], F32, tag="m1")
                    t2 = p_work.tile([128, 512], F32, tag="m2")
                    for hh in range(2):
                        sl = slice(hh * 512, (hh + 1) * 512)
                        t3 = p_work.tile([128, 512], F32, tag="m3")
                        t4 = p_work.tile([128, 512], F32, tag="m4")
                        nc.vector.tensor_mul(t1[:], br[:, sl], ct["TWr"][:])
                        nc.vector.tensor_mul(t2[:], bi[:, sl], ct["TWi"][:])
                        nc.vector.tensor_mul(t3[:], br[:, sl], ct["TWi"][:])
                        nc.vector.tensor_mul(t4[:], bi[:, sl], ct["TWr"][:])
                        nc.gpsimd.tensor_sub(cr[:, sl], t1[:], t2[:])
                        nc.gpsimd.tensor_add(ci[:, sl], t3[:], t4[:])
                    # step3: X[(cc,k1), (pr, k2)]
                    xr = ps_big.tile([128, 1024], F32, tag="bigps")
                    xi = ps_big.tile([128, 1024], F32, tag="bigps")
                    for p in range(NPAIR):
                        sl = slice(p * 128, (p + 1) * 128)
                        nc.tensor.matmul(xr[:, sl], ct["F2c"][:], cr[:, sl],
                                         start=True, stop=False)
                        nc.tensor.matmul(xr[:, sl], ct["F2s"][:], ci[:, sl],
                                         start=False, stop=True)
                        nc.tensor.matmul(xi[:, sl], ct["F2c"][:], ci[:, sl],
                                         start=True, stop=False)
                        nc.tensor.matmul(xi[:, sl], ct["F2sn"][:], cr[:, sl],
                                         start=False, stop=True)
                    return xr, xi

                for ch in range(N_CHUNK):
                    # filter chunk spectrum -> K (bf16)
                    kxr, kxi = fwd_fft(ff4, ch, "f")
                    kr = p_work.tile([128, 1024], BF16, tag="Kr")
                    ki = p_work.tile([128, 1024], BF16, tag="Ki")
                    nc.scalar.copy(out=kr[:], in_=kxr[:])
                    nc.scalar.copy(out=ki[:], in_=kxi[:])
                    # u chunk spectrum (stays in psum)
                    uxr, uxi = fwd_fft(uf4, ch, "u")
                    # spectrum multiply: Y = U * K
                    yr = p_work.tile([128, 1024], BF16, tag="Yr")
                    yi = p_work.tile([128, 1024], BF16, tag="Yi")
                    m1 = p_work.tile([128, 1024], F32, tag="m1")
                    m2 = p_work.tile([128, 1024], F32, tag="m2")
                    m3 = p_work.tile([128, 1024], F32, tag="m3")
                    m4 = p_work.tile([128, 1024], F32, tag="m4")
                    nc.vector.tensor_mul(m1[:], uxr[:], kr[:])
                    nc.vector.tensor_mul(m2[:], uxi[:], ki[:])
                    nc.vector.tensor_mul(m3[:], uxr[:], ki[:])
                    nc.vector.tensor_mul(m4[:], uxi[:], kr[:])
                    nc.gpsimd.tensor_sub(yr[:], m1[:], m2[:])
                    nc.gpsimd.tensor_add(yi[:], m3[:], m4[:])
                    # inverse stepA: Z[(cc,s), (pr, k2)]
                    zr = ps_big.tile([128, 1024], F32, tag="bigps")
                    zi = ps_big.tile([128, 1024], F32, tag="bigps")
                    for p in range(NPAIR):
                        sl = slice(p * 128, (p + 1) * 128)
                        nc.tensor.matmul(zr[:, sl], ct["V1c"][:], yr[:, sl],
                                         start=True, stop=False)
                        nc.tensor.matmul(zr[:, sl], ct["V1sn"][:], yi[:, sl],
                                         start=False, stop=True)
                        nc.tensor.matmul(zi[:, sl], ct["V1c"][:], yi[:, sl],
                                         start=True, stop=False)
                        nc.tensor.matmul(zi[:, sl], ct["V1s"][:], yr[:, sl],
                                         start=False, stop=True)
                    # inverse twiddle (incl 1/N): Z2 = Z * W^{-s k2}/N
                    z2r = p_work.tile([128, 1024], BF16, tag="Z2r")
                    z2i = p_work.tile([128, 1024], BF16, tag="Z2i")
                    for hh in range(2):
                        sl = slice(hh * 512, (hh + 1) * 512)
                        w1t = p_work.tile([128, 512], F32, tag="m1")
                        w2t = p_work.tile([128, 512], F32, tag="m2")
                        w3t = p_work.tile([128, 512], F32, tag="m3")
                        w4t = p_work.tile([128, 512], F32, tag="m4")
                        nc.vector.tensor_mul(w1t[:], zr[:, sl], ct["TIr"][:])
                        nc.vector.tensor_mul(w2t[:], zi[:, sl], ct["TIi"][:])
                        nc.vector.tensor_mul(w3t[:], zr[:, sl], ct["TIi"][:])
                        nc.vector.tensor_mul(w4t[:], zi[:, sl], ct["TIr"][:])
                        nc.gpsimd.tensor_sub(z2r[:, sl], w1t[:], w2t[:])
                        nc.gpsimd.tensor_add(z2i[:, sl], w3t[:], w4t[:])
                    # transpose Z2 pairs -> ZT [k2, (pr, cc, s)]
                    ztp_r = ps_big.tile([128, 1024], BF16, tag="bigps")
                    ztp_i = ps_big.tile([128, 1024], BF16, tag="bigps")
                    for p in range(NPAIR):
                        sl = slice(p * 128, (p + 1) * 128)
                        nc.tensor.transpose(ztp_r[:, sl], z2r[:, sl], ct["IDT"][:])
                        nc.tensor.transpose(ztp_i[:, sl], z2i[:, sl], ct["IDT"][:])
                    ztr = p_work.tile([128, 1024], BF16, tag="ZTsr")
                    zti = p_work.tile([128, 1024], BF16, tag="ZTsi")
                    nc.scalar.copy(out=ztr[:], in_=ztp_r[:])
                    nc.scalar.copy(out=zti[:], in_=ztp_i[:])
                    # stepB: y[q, (pr, cc, s)] real part
                    yps = ps_small.tile([64, 1024], F32, tag="smps")
                    for p in range(NPAIR):
                        sl = slice(p * 128, (p + 1) * 128)
                        nc.tensor.matmul(yps[:, sl], ct["V2c"][:], ztr[:, sl],
                                         start=True, stop=False)
                        nc.tensor.matmul(yps[:, sl], ct["V2sn"][:], zti[:, sl],
                                         start=False, stop=True)
                    # gate: out = y * proj + u   (iteration order (c16, s64))
                    cch0 = ch * CHUNK_C
                    prj = projb[:].rearrange("p (s c) -> p c s", s=64)[
                        :, cch0 : cch0 + CHUNK_C, :
                    ]
                    # residual in (pr, cc, s) iteration order
                    ures = uf4[:, ch * NPAIR : (ch + 1) * NPAIR, :, :]
                    gt = p_work.tile([64, 1024], F32, tag="gt")
                    och = p_work.tile([64, 1024], BF16, tag="och")
                    nc.vector.tensor_mul(gt[:], yps[:], prj)
                    nc.gpsimd.tensor_add(
                        och[:].rearrange("p (s c) -> p c s", s=64), gt[:], ures
                    )
                    nc.sync.dma_start(
                        out=out.rearrange("(q s) c -> q s c", s=64)[
                            :, :, c0 + cch0 : c0 + cch0 + CHUNK_C
                        ],
                        in_=och[:].rearrange("p (s c) -> p s c", s=64),
                    )

    _split_multi_waits(nc)
    _NC_CACHE["nc"] = nc
    return nc


# ---------------- host side ----------------


def _to_bf16(x):
    return np.asarray(x, dtype=np.float32).astype(NPBF)


def _host_prep(inputs):
    """Returns per-core input dicts (numpy, bf16 where applicable)."""
    u = np.asarray(inputs["u"], dtype=np.float32)
    z = np.asarray(inputs["z"], dtype=np.float32)
    w1 = np.asarray(inputs["w1"], dtype=np.float32)
    b1 = np.asarray(inputs["b1"], dtype=np.float32)
    w2 = np.asarray(inputs["w2"], dtype=np.float32)
    b2 = np.asarray(inputs["b2"], dtype=np.float32)
    pw = np.asarray(inputs["pw"], dtype=np.float32)
    pb = np.asarray(inputs["pb"], dtype=np.float32)

    pe = z[0, :L]                                     # (L, 3)
    h = np.maximum(pe @ w1.T + b1, 0.0)               # (L, 16)
    hT = np.vstack([h.T, np.ones((1, L), np.float32)])  # (17, L)
    hT_bf = _to_bf16(hT)

    pwT = _to_bf16(pw.T)                              # (D, D)
    w2b = np.vstack([w2.T, b2[None]])                 # (17, D)
    w2b_bf = _to_bf16(w2b)
    pbh_bf = _to_bf16(0.5 * pb[None])                 # (1, D), pb/2
    u_bf = _to_bf16(u)                                # (B, L, D)

    in_maps = []
    for c in range(N_CORES):
        b, hf = c // 2, c % 2
        sl = slice(hf * HALF, (hf + 1) * HALF)
        in_maps.append(
            {
                "ub": np.ascontiguousarray(u_bf[b][:, sl]),
                "pwh": np.ascontiguousarray(pwT[sl, :]),
                "hT": hT_bf,
                "w2b": np.ascontiguousarray(w2b_bf[:, sl]),
                "pbh": pbh_bf,
            }
        )
    return in_maps


_RUN_CACHE = {}


def _get_runner():
    if "run" in _RUN_CACHE:
        return _RUN_CACHE["run"]
    import jax
    import jax.numpy as jnp
    from jax.sharding import Mesh, PartitionSpec
    from jax.experimental.shard_map import shard_map
    from concourse.bass2jax import (
        _bass_exec_p,
        install_neuronx_cc_hook,
        partition_id_tensor,
    )

    nc = _build_nc()
    install_neuronx_cc_hook()

    partition_name = (
        nc.partition_id_tensor.name if nc.partition_id_tensor else None
    )
    in_names = []
    out_names = []
    out_avals = []
    for alloc in nc.m.functions[0].allocations:
        if not isinstance(alloc, mybir.MemoryLocationSet):
            continue
        name = alloc.memorylocations[0].name
        if alloc.kind == "ExternalInput":
            if name == partition_name:
                continue
            in_names.append(name)
        elif alloc.kind == "ExternalOutput":
            out_names.append(name)
            out_avals.append(
                jax.core.ShapedArray(
                    tuple(alloc.tensor_shape), mybir.dt.np(alloc.dtype)
                )
            )
    n_params = len(in_names)
    bind_in_names = tuple(in_names) + tuple(out_names)
    if partition_name is not None:
        bind_in_names = bind_in_names + (partition_name,)

    def _body(*args):
        operands = list(args)
        if partition_name is not None:
            operands.append(partition_id_tensor())
        outs = _bass_exec_p.bind(
            *operands,
            out_avals=tuple(out_avals),
            in_names=bind_in_names,
            out_names=tuple(out_names),
            lowering_input_output_aliases=(),
            sim_require_finite=True,
            sim_require_nnan=True,
            nc=nc,
        )
        return tuple(outs)

    devices = jax.devices()[:N_CORES]
    mesh = Mesh(np.asarray(devices), ("core",))
    n_outs = len(out_names)
    in_specs = (PartitionSpec("core"),) * (n_params + n_outs)
    out_specs = (PartitionSpec("core"),) * n_outs
    sharded = jax.jit(
        shard_map(
            _body, mesh=mesh, in_specs=in_specs, out_specs=out_specs,
            check_rep=False,
        ),
        keep_unused=True,
    )
    # device-resident zero buffers for the ExternalOutput operands: put
    # ONCE and reused every call (not donated, so they stay valid).
    from jax.sharding import NamedSharding

    zsh = NamedSharding(mesh, PartitionSpec("core"))
    zero_outs = [
        jax.device_put(
            np.zeros((N_CORES * av.shape[0], *av.shape[1:]), av.dtype), zsh
        )
        for av in out_avals
    ]
    jax.block_until_ready(zero_outs)
    _RUN_CACHE["run"] = (sharded, in_names, out_names, zero_outs)
    return _RUN_CACHE["run"]


def kernel(**inputs):
    in_maps = _host_prep(inputs)
    sharded, in_names, out_names, zero_outs = _get_runner()
    concat_in = [
        np.concatenate([in_maps[c][nm] for c in range(N_CORES)], axis=0)
        for nm in in_names
    ]
    out_arrs = sharded(*concat_in, *zero_outs)
    res = np.asarray(out_arrs[0]).reshape(N_CORES, L, HALF)

    out = np.empty((B, L, D), dtype=np.float32)
    for c in range(N_CORES):
        b, hf = c // 2, c % 2
        out[b, :, hf * HALF : (hf + 1) * HALF] = res[c].astype(np.float32)
    return out
